# revision 10
# baseline (speedup 1.0000x reference)
"""Conditional contrastive loss on 8 TRN2 NeuronCores (Bass/Tile).

Strategy (data-parallel over rows, per sharding hint):
  - Each core owns 512 rows (of 4096) of inst_embed ("x") and proxy ("p").
  - Host does all O(N*D) prep: row-normalizes x and p (exactly as the
    reference: x / max(||x||, 1e-8)), scales by S=8, quantizes to fp8-e4m3,
    and lays the data out matmul-native (chunk-major [128, KC, n] so the
    tensor engine can run DoubleRow fp8 matmuls with K=256 per instruction).
    It also pre-gathers each core's positive-mask rows negative_mask[labels]
    (fp8 - 0/1 is exact).
  - Device per core: 16 output tiles ([128 rows, 2048 cols]); each tile is
    8 DoubleRow fp8 matmuls accumulated in PSUM (4 banks), then
    exp((sim - margin)/T) on the scalar engine straight out of PSUM with
    accum_out = row sums -> denominator for free (the scale folds away the
    S^2 from quantization); z written to SBUF in bf16.
  - numerator = scalar_tensor_tensor(z * mask) on DVE with accum_out
    (walrus rejects TensorScalarPtr on Pool, so all 16 stay on DVE; DVE
    is the 2.2us/tile steady-state pace-setter).
  - Descriptor generation (DIRECT2D, ~0.65us each, serial per sequencer)
    is split: lhsT/xn8/out on the Sync sequencer, the four mask loads on
    the otherwise-idle GpSimd sequencer, so the first mask is in SBUF
    before the first z tile is.
  - A zeros-matmul HAM warm-up runs at t~0 (no DMA dependency) so the PE
    is at full clock roughly when the first real matmul's data lands.
  - Device emits ln(den), ln(num) per row for both matrices as one
    [128, 16] tile -> single contiguous output DMA; the host does the
    final O(N) mean + gather across cores.
"""
import numpy as np
import ml_dtypes

import concourse.bacc as bacc
import concourse.tile as tile
from concourse import mybir, bass_utils

N_FULL = 4096
D = 512
C = 100
N_CORES = 8
RP = N_FULL // N_CORES  # rows per core = 512
P = 128                 # SBUF partitions
KC = D // P             # contraction chunks = 4
NPAIR = KC // 2         # DoubleRow K-pairs = 2
JT = 512                # columns per PSUM bank
JG = 2048               # columns per PSUM group (4 banks)
NG = N_FULL // JG       # groups per (i-tile, matrix) = 2
IT = RP // P            # i-tiles per core = 4
SCALE = 8.0             # fp8 quantization scale on normalized rows
WARMUP_MM = 9           # zeros matmuls to lift the HAM clock gate

F32 = mybir.dt.float32
BF16 = mybir.dt.bfloat16
FP8 = mybir.dt.float8e4
AF = mybir.ActivationFunctionType
ALU = mybir.AluOpType
AX = mybir.AxisListType
DR = mybir.MatmulPerfMode.DoubleRow

_CACHE = {}


def _pin_act_table_set():
    """This kernel only uses Exp and Ln. Left alone, bacc's table chooser
    picks exp_and_others for Exp and natural_log for Ln, paying a ~2.7us
    ACT table reload on every Ln<->Exp transition. Blank out every set
    except natural_log_exp_and_others (indices preserved) so both
    functions resolve to the one set -> a single load at startup."""
    from concourse import hw_specs
    orig = hw_specs.get_activation_tables

    def patched(arch):
        tabs = orig(arch)
        return {name: (fns if name == "natural_log_exp_and_others" else set())
                for name, fns in tabs.items()}

    bacc.get_activation_tables = patched


def _build(inv_t: float, bias_den: float):
    _pin_act_table_set()
    nc = bacc.Bacc("TRN2", target_bir_lowering=False, debug=False,
                   num_devices=N_CORES)

    # chunk-major fp8: [kp, kc, j] = xn[j, kc*P + kp] * SCALE
    xn8 = nc.dram_tensor("xn8", [P, KC, N_FULL], FP8, kind="ExternalInput")
    # per-core lhsT rows: [:, 0:KC, :] = proxy rows, [:, KC:2KC, :] = x rows
    lhs8 = nc.dram_tensor("lhs8", [P, 2 * KC, RP], FP8, kind="ExternalInput")
    mk = nc.dram_tensor("mk", [RP, N_FULL], FP8, kind="ExternalInput")
    out = nc.dram_tensor("out", [P, 4 * IT], F32, kind="ExternalOutput")

    with tile.TileContext(nc) as tc:
        with (
            tc.tile_pool(name="xpool", bufs=1) as xpool,
            tc.tile_pool(name="lhs", bufs=1) as lhs,
            tc.tile_pool(name="zpool", bufs=3) as zpool,
            tc.tile_pool(name="maskp", bufs=1) as maskp,
            tc.tile_pool(name="small", bufs=1) as small,
            tc.tile_pool(name="ps", bufs=2, space="PSUM") as pspool,
        ):
            # ---- constants (no DMA dependency) ----
            zeros_l = small.tile([P, P], BF16, name="zeros_l")
            nc.vector.memset(zeros_l[:], 0.0)
            zeros_r = small.tile([P, JT], BF16, name="zeros_r")
            nc.vector.memset(zeros_r[:], 0.0)
            bias_t = small.tile([P, 1], F32, name="bias_t")
            nc.vector.memset(bias_t[:], bias_den)

            # ---- loads; descriptor gen split across sequencers ----
            lhs8_t = lhs.tile([P, 2 * KC, RP], FP8, name="lhs8_t")
            nc.sync.dma_start(lhs8_t[:], lhs8.ap())
            xn8_t = xpool.tile([P, KC, N_FULL], FP8, name="xn8_t")
            for g in range(NG):
                nc.sync.dma_start(xn8_t[:, :, g * JG:(g + 1) * JG],
                                  xn8.ap()[:, :, g * JG:(g + 1) * JG])
            mask_t = []
            for it in range(IT):
                t = maskp.tile([P, N_FULL], FP8, name=f"mask{it}")
                nc.gpsimd.dma_start(t[:], mk.ap()[it * P:(it + 1) * P, :])
                mask_t.append(t)

            # ---- accumulators: one [P, 16, NG] tile (dim1 = it*4+mat*2+kind,
            # kind 0=den 1=num) so a single strided reduce at the end
            # collapses the g axis with no mid-pipeline FIFO blocking.
            acc = small.tile([P, 4 * IT, NG], F32, name="acc")
            sums = small.tile([P, 4 * IT], F32, name="sums")
            lns = small.tile([P, 4 * IT], F32, name="lns")

            def acc_col(it, mat, kind, g):
                c = it * 4 + mat * 2 + kind
                return acc[:, c, g:g + 1]

            # ---- main loop (group-major) ----
            for g in range(NG):
                for it in range(IT):
                    i0 = it * P
                    for mat in range(2):
                        ps = pspool.tile([P, JG], F32,
                                         name=f"ps_{it}_{mat}_{g}", tag="ps")
                        first_tile = (g == 0 and it == 0 and mat == 0)
                        if first_tile:
                            # HAM warm-up: accumulate exact zeros into bank 0
                            # starting at t~0 (zeros live in SBUF via memset,
                            # no DMA dependency) so the PE is near full clock
                            # when the real stream begins.
                            for w in range(WARMUP_MM):
                                nc.tensor.matmul(
                                    ps[:, 0:JT], zeros_l[:], zeros_r[:],
                                    start=(w == 0), stop=False,
                                )
                        for pr in range(NPAIR):
                            for jl in range(JG // JT):
                                j0 = g * JG + jl * JT
                                kc0 = mat * KC + 2 * pr
                                nc.tensor.matmul(
                                    ps[:, jl * JT:(jl + 1) * JT],
                                    lhs8_t[:, kc0:kc0 + 2, i0:i0 + P],
                                    xn8_t[:, 2 * pr:2 * pr + 2, j0:j0 + JT],
                                    start=(pr == 0 and not (first_tile and jl == 0)),
                                    stop=(pr == NPAIR - 1),
                                    perf_mode=DR,
                                )
                        z = zpool.tile([P, JG], BF16,
                                       name=f"z_{it}_{mat}_{g}", tag="z")
                        zo = zpool.tile([P, JG], BF16,
                                        name=f"zo_{it}_{mat}_{g}", tag="zo",
                                        bufs=2)
                        nc.scalar.activation(
                            z[:], ps[:], AF.Exp,
                            bias=bias_t[:], scale=inv_t / (SCALE * SCALE),
                            accum_out=acc_col(it, mat, 0, g),
                        )
                        nc.vector.scalar_tensor_tensor(
                            out=zo[:], in0=z[:], scalar=1.0,
                            in1=mask_t[it][:, g * JG:(g + 1) * JG],
                            op0=ALU.mult, op1=ALU.mult,
                            accum_out=acc_col(it, mat, 1, g),
                        )

            # ---- tail: collapse g, ln, one contiguous out DMA ----
            nc.vector.tensor_reduce(sums[:], acc[:], AX.X, ALU.add)
            nc.scalar.activation(lns[:], sums[:], AF.Ln)
            nc.sync.dma_start(out.ap(), lns[:])

    nc.compile()
    return nc


def make_in_maps(x, p, nmf, lab):
    eps = 1e-8
    xn = x / np.maximum(np.linalg.norm(x, axis=-1, keepdims=True), eps)
    pn = p / np.maximum(np.linalg.norm(p, axis=-1, keepdims=True), eps)
    f8 = ml_dtypes.float8_e4m3
    # chunk-major [P, KC, n]: [kp, kc, j] = v[j, kc*P + kp]
    xn8 = np.ascontiguousarray(
        (xn.T * SCALE).astype(f8).reshape(KC, P, N_FULL).transpose(1, 0, 2))
    pn8 = np.ascontiguousarray(
        (pn.T * SCALE).astype(f8).reshape(KC, P, N_FULL).transpose(1, 0, 2))
    in_maps = []
    for c in range(N_CORES):
        rows = slice(c * RP, (c + 1) * RP)
        lhs8 = np.concatenate([pn8[:, :, rows], xn8[:, :, rows]], axis=1)
        in_maps.append({
            "xn8": xn8,
            "lhs8": np.ascontiguousarray(lhs8),
            "mk": nmf[lab[rows]].astype(f8),
        })
    return in_maps


def kernel(inst_embed, proxy, negative_mask, labels, temperature, margin):
    t = float(np.asarray(temperature))
    m = float(np.asarray(margin))
    inv_t = 1.0 / t
    bias_den = -m / t

    key = (t, m)
    if key not in _CACHE:
        _CACHE[key] = _build(inv_t, bias_den)
    nc = _CACHE[key]

    x = np.asarray(inst_embed, dtype=np.float32)
    p = np.asarray(proxy, dtype=np.float32)
    nmf = np.asarray(negative_mask, dtype=np.float32)
    lab = np.asarray(labels).astype(np.int64)

    in_maps = make_in_maps(x, p, nmf, lab)

    res = bass_utils.run_bass_kernel_spmd(nc, in_maps,
                                          core_ids=list(range(N_CORES)))
    # out[p, it*4 + q] holds row it*128+p of the core's block; q =
    # [ln den_p2i, ln num_p2i, ln den_i2i, ln num_i2i]
    parts = []
    for c in range(N_CORES):
        o = res.results[c]["out"].reshape(P, IT, 4).transpose(1, 0, 2)
        parts.append(o.reshape(RP, 4))
    outs = np.concatenate(parts, axis=0)
    ld_p, ln_p, ld_i, ln_i = (outs[:, q].astype(np.float64) for q in range(4))
    loss = (-2.0 * np.log(t)
            + (ld_p - ln_p).mean()
            + (ld_i - ln_i).mean())
    return np.float32(loss)


# revision 14
# speedup vs baseline: 1.0752x; 1.0752x over previous
"""Conditional contrastive loss on 8 TRN2 NeuronCores (Bass/Tile).

Strategy (data-parallel over rows, per sharding hint):
  - Each core owns 512 rows (of 4096) of inst_embed ("x") and proxy ("p").
  - Host does all O(N*D) prep: row-normalizes x and p (exactly as the
    reference: x / max(||x||, 1e-8)), scales by S=8, quantizes to fp8-e4m3,
    and lays the data out matmul-native (chunk-major [128, KC, n] so the
    tensor engine can run DoubleRow fp8 matmuls with K=256 per instruction).
    It also pre-gathers each core's positive-mask rows negative_mask[labels]
    (fp8 - 0/1 is exact).
  - Device per core: 16 output tiles ([128 rows, 2048 cols]); each tile is
    8 DoubleRow fp8 matmuls accumulated in PSUM (4 banks), then
    exp((sim - margin)/T) on the scalar engine straight out of PSUM with
    accum_out = row sums -> denominator for free (the scale folds away the
    S^2 from quantization); z written to SBUF in bf16.
  - numerator = scalar_tensor_tensor(z * mask) on DVE with accum_out
    (walrus rejects TensorScalarPtr on Pool, so all 16 stay on DVE; DVE
    is the 2.2us/tile steady-state pace-setter).
  - Descriptor generation (DIRECT2D, ~0.65us each, serial per sequencer)
    is split: lhsT/xn8/out on the Sync sequencer, the four mask loads on
    the otherwise-idle GpSimd sequencer, so the first mask is in SBUF
    before the first z tile is.
  - A zeros-matmul HAM warm-up runs at t~0 (no DMA dependency) so the PE
    is at full clock roughly when the first real matmul's data lands.
  - Device emits ln(den), ln(num) per row for both matrices as one
    [128, 16] tile -> single contiguous output DMA; the host does the
    final O(N) mean + gather across cores.
"""
import numpy as np
import ml_dtypes

import concourse.bacc as bacc
import concourse.tile as tile
from concourse import mybir, bass_utils

N_FULL = 4096
D = 512
C = 100
N_CORES = 8
RP = N_FULL // N_CORES  # rows per core = 512
P = 128                 # SBUF partitions
KC = D // P             # contraction chunks = 4
NPAIR = KC // 2         # DoubleRow K-pairs = 2
JT = 512                # columns per PSUM bank
JG = 2048               # columns per PSUM group (4 banks)
NG = N_FULL // JG       # groups per (i-tile, matrix) = 2
IT = RP // P            # i-tiles per core = 4
SCALE = 8.0             # fp8 quantization scale on normalized rows
WARMUP_MM = 7           # zeros matmuls to lift the HAM clock gate

F32 = mybir.dt.float32
BF16 = mybir.dt.bfloat16
FP8 = mybir.dt.float8e4
AF = mybir.ActivationFunctionType
ALU = mybir.AluOpType
AX = mybir.AxisListType
DR = mybir.MatmulPerfMode.DoubleRow

_CACHE = {}


def _pin_act_table_set():
    """This kernel only uses Exp and Ln. Left alone, bacc's table chooser
    picks exp_and_others for Exp and natural_log for Ln, paying a ~2.7us
    ACT table reload on every Ln<->Exp transition. Blank out every set
    except natural_log_exp_and_others (indices preserved) so both
    functions resolve to the one set -> a single load at startup."""
    from concourse import hw_specs
    orig = hw_specs.get_activation_tables

    def patched(arch):
        tabs = orig(arch)
        return {name: (fns if name == "natural_log_exp_and_others" else set())
                for name, fns in tabs.items()}

    bacc.get_activation_tables = patched


def _build(inv_t: float, bias_den: float):
    _pin_act_table_set()
    nc = bacc.Bacc("TRN2", target_bir_lowering=False, debug=False,
                   num_devices=N_CORES)

    # chunk-major fp8: [kp, kc, j] = xn[j, kc*P + kp] * SCALE
    xn8 = nc.dram_tensor("xn8", [P, KC, N_FULL], FP8, kind="ExternalInput")
    # per-core lhsT rows: [:, 0:KC, :] = proxy rows, [:, KC:2KC, :] = x rows
    lhs8 = nc.dram_tensor("lhs8", [P, 2 * KC, RP], FP8, kind="ExternalInput")
    mk = nc.dram_tensor("mk", [RP, N_FULL], BF16, kind="ExternalInput")
    out = nc.dram_tensor("out", [P, 4 * IT], F32, kind="ExternalOutput")

    with tile.TileContext(nc) as tc:
        with (
            tc.tile_pool(name="xpool", bufs=1) as xpool,
            tc.tile_pool(name="lhs", bufs=1) as lhs,
            tc.tile_pool(name="zpool", bufs=3) as zpool,
            tc.tile_pool(name="maskp", bufs=1) as maskp,
            tc.tile_pool(name="small", bufs=1) as small,
            tc.tile_pool(name="ps", bufs=2, space="PSUM") as pspool,
        ):
            # ---- constants (no DMA dependency) ----
            zeros_l = small.tile([P, P], BF16, name="zeros_l")
            nc.vector.memset(zeros_l[:], 0.0)
            zeros_r = small.tile([P, JT], BF16, name="zeros_r")
            nc.vector.memset(zeros_r[:], 0.0)
            bias_t = small.tile([P, 1], F32, name="bias_t")
            nc.vector.memset(bias_t[:], bias_den)

            # ---- loads, all on the Sync sequencer whose strict-FIFO
            # DIRECT2D order (~0.65us each) sets the DMA priority: first
            # everything the g=0 tiles need, in consumption order, then g=1.
            lhs8_t = lhs.tile([P, 2 * KC, RP], FP8, name="lhs8_t")
            xn8_t = xpool.tile([P, KC, N_FULL], FP8, name="xn8_t")
            mask_t = [maskp.tile([P, N_FULL], BF16, name=f"mask{it}")
                      for it in range(IT)]
            nc.sync.dma_start(lhs8_t[:], lhs8.ap())
            for g in range(NG):
                for pr in range(NPAIR):
                    nc.sync.dma_start(
                        xn8_t[:, 2 * pr:2 * pr + 2, g * JG:(g + 1) * JG],
                        xn8.ap()[:, 2 * pr:2 * pr + 2, g * JG:(g + 1) * JG])
                for it in range(IT):
                    nc.sync.dma_start(
                        mask_t[it][:, g * JG:(g + 1) * JG],
                        mk.ap()[it * P:(it + 1) * P, g * JG:(g + 1) * JG])

            # ---- accumulators: one [P, 16, NG] tile (dim1 = it*4+mat*2+kind,
            # kind 0=den 1=num) so a single strided reduce at the end
            # collapses the g axis with no mid-pipeline FIFO blocking.
            acc = small.tile([P, 4 * IT, NG], F32, name="acc")
            sums = small.tile([P, 4 * IT], F32, name="sums")
            lns = small.tile([P, 4 * IT], F32, name="lns")

            def acc_col(it, mat, kind, g):
                c = it * 4 + mat * 2 + kind
                return acc[:, c, g:g + 1]

            # ---- main loop (group-major) ----
            for g in range(NG):
                for it in range(IT):
                    i0 = it * P
                    for mat in range(2):
                        ps = pspool.tile([P, JG], F32,
                                         name=f"ps_{it}_{mat}_{g}", tag="ps")
                        first_tile = (g == 0 and it == 0 and mat == 0)
                        if first_tile:
                            # HAM warm-up: accumulate exact zeros into bank 0
                            # starting at t~0 (zeros live in SBUF via memset,
                            # no DMA dependency) so the PE is near full clock
                            # when the real stream begins.
                            for w in range(WARMUP_MM):
                                nc.tensor.matmul(
                                    ps[:, 0:JT], zeros_l[:], zeros_r[:],
                                    start=(w == 0), stop=False,
                                )
                        for pr in range(NPAIR):
                            for jl in range(JG // JT):
                                j0 = g * JG + jl * JT
                                kc0 = mat * KC + 2 * pr
                                nc.tensor.matmul(
                                    ps[:, jl * JT:(jl + 1) * JT],
                                    lhs8_t[:, kc0:kc0 + 2, i0:i0 + P],
                                    xn8_t[:, 2 * pr:2 * pr + 2, j0:j0 + JT],
                                    start=(pr == 0 and not (first_tile and jl == 0)),
                                    stop=(pr == NPAIR - 1),
                                    perf_mode=DR,
                                )
                        z = zpool.tile([P, JG], BF16,
                                       name=f"z_{it}_{mat}_{g}", tag="z")
                        zo = zpool.tile([P, JG], BF16,
                                        name=f"zo_{it}_{mat}_{g}", tag="zo",
                                        bufs=2)
                        nc.scalar.activation(
                            z[:], ps[:], AF.Exp,
                            bias=bias_t[:], scale=inv_t / (SCALE * SCALE),
                            accum_out=acc_col(it, mat, 0, g),
                        )
                        nc.vector.scalar_tensor_tensor(
                            out=zo[:], in0=z[:], scalar=1.0,
                            in1=mask_t[it][:, g * JG:(g + 1) * JG],
                            op0=ALU.mult, op1=ALU.mult,
                            accum_out=acc_col(it, mat, 1, g),
                        )

            # ---- tail: collapse g, ln, one contiguous out DMA ----
            nc.vector.tensor_reduce(sums[:], acc[:], AX.X, ALU.add)
            nc.scalar.activation(lns[:], sums[:], AF.Ln)
            nc.sync.dma_start(out.ap(), lns[:])

    nc.compile()
    return nc


def make_in_maps(x, p, nmf, lab):
    eps = 1e-8
    xn = x / np.maximum(np.linalg.norm(x, axis=-1, keepdims=True), eps)
    pn = p / np.maximum(np.linalg.norm(p, axis=-1, keepdims=True), eps)
    f8 = ml_dtypes.float8_e4m3
    # chunk-major [P, KC, n]: [kp, kc, j] = v[j, kc*P + kp]
    xn8 = np.ascontiguousarray(
        (xn.T * SCALE).astype(f8).reshape(KC, P, N_FULL).transpose(1, 0, 2))
    pn8 = np.ascontiguousarray(
        (pn.T * SCALE).astype(f8).reshape(KC, P, N_FULL).transpose(1, 0, 2))
    in_maps = []
    for c in range(N_CORES):
        rows = slice(c * RP, (c + 1) * RP)
        lhs8 = np.concatenate([pn8[:, :, rows], xn8[:, :, rows]], axis=1)
        in_maps.append({
            "xn8": xn8,
            "lhs8": np.ascontiguousarray(lhs8),
            "mk": nmf[lab[rows]].astype(ml_dtypes.bfloat16),
        })
    return in_maps


def kernel(inst_embed, proxy, negative_mask, labels, temperature, margin):
    t = float(np.asarray(temperature))
    m = float(np.asarray(margin))
    inv_t = 1.0 / t
    bias_den = -m / t

    key = (t, m)
    if key not in _CACHE:
        _CACHE[key] = _build(inv_t, bias_den)
    nc = _CACHE[key]

    x = np.asarray(inst_embed, dtype=np.float32)
    p = np.asarray(proxy, dtype=np.float32)
    nmf = np.asarray(negative_mask, dtype=np.float32)
    lab = np.asarray(labels).astype(np.int64)

    in_maps = make_in_maps(x, p, nmf, lab)

    res = bass_utils.run_bass_kernel_spmd(nc, in_maps,
                                          core_ids=list(range(N_CORES)))
    # out[p, it*4 + q] holds row it*128+p of the core's block; q =
    # [ln den_p2i, ln num_p2i, ln den_i2i, ln num_i2i]
    parts = []
    for c in range(N_CORES):
        o = res.results[c]["out"].reshape(P, IT, 4).transpose(1, 0, 2)
        parts.append(o.reshape(RP, 4))
    outs = np.concatenate(parts, axis=0)
    ld_p, ln_p, ld_i, ln_i = (outs[:, q].astype(np.float64) for q in range(4))
    loss = (-2.0 * np.log(t)
            + (ld_p - ln_p).mean()
            + (ld_i - ln_i).mean())
    return np.float32(loss)
